# revision 1
# baseline (speedup 1.0000x reference)
"""Trainium2 Bass kernel for nn_Cross_Attention (B=16, C=256, H=W=96).

reference:
    q = Z1.reshape(B, C, N); k = Zr.reshape(B, C, N)         # N = H*W
    energy    = q @ k^T                                       # [B, C, C]
    attention = softmax(rowmax(energy) - energy, axis=-1)
    out       = attention @ k                                 # [B, C, N]
    return beta * out + Zr

Strategy: data-parallel over batch, 2 batches per NeuronCore on 8 cores.
Uploads per core: q^T in bf16 (host pre-packed [P, NT, C] partition-major so
the N-contraction matmul streams straight from DRAM) and Zr in f32.  k is
derived on-chip: kb = bf16(Zr) (ScalarE downcast) feeds the second matmul
directly and is transposed tile-by-tile on the TensorE (transpose-mode
matmul) into k^T tiles for the energy matmul — so k crosses HBM once.
softmax(max - e) == exp(min - e) / sum(exp(min - e)) row-wise: only a
row-min is needed, exp args are always <= 0 (no overflow), the sum is >= 1
(no div-by-0).  beta and 1/sum are folded into the attention weights before
the second matmul, so the final blend is a single add with the f32-resident
Zr (bitwise-exact output when beta == 0).
"""

from contextlib import ExitStack

import ml_dtypes
import numpy as np

import concourse.bass as bass
import concourse.tile as tile
from concourse import bacc, mybir
from concourse.bass_utils import run_bass_kernel_spmd
from concourse.masks import make_identity

B, C, H, W = 16, 256, 96, 96
N = H * W                    # 9216
P = 128
NCORES = 8
BL = B // NCORES             # 2 batches per core
CT = C // P                  # 2 c-tiles of 128
NT = N // P                  # 72 contraction tiles for energy
TCH = 18                     # qt tiles per DMA chunk
NCH = NT // TCH              # 4 chunks (last one split per c-tile)
TQT = (NCH - 1) * TCH        # 54 t-tiles in the interleaved qt tensor
NH = N // 2                  # 4608: kb slice width (half a c-tile row)
NQ = N // 4                  # 2304: zr tile width (quarter c-tile row)
TPH = NH // P                # 36 n-tiles per h-half
OW = 384                     # mm2 psum chunk width (6 per zr quarter)
WPH = NH // OW               # 12 psum chunks per h-half

F32 = mybir.dt.float32
BF16 = mybir.dt.bfloat16


def _build_program():
    nc = bacc.Bacc("TRN2", target_bir_lowering=False, debug=False,
                   num_devices=NCORES)

    qt_ext = nc.dram_tensor("qt", [BL, P, TQT, C], BF16, kind="ExternalInput")
    qtt_ext = nc.dram_tensor("qtt", [BL, CT, P, TCH, P], BF16,
                             kind="ExternalInput")
    zr_ext = nc.dram_tensor("zr", [BL, C, N], F32, kind="ExternalInput")
    beta_ext = nc.dram_tensor("beta", [1], F32, kind="ExternalInput")
    out_ext = nc.dram_tensor("out", [BL, C, N], F32, kind="ExternalOutput")

    with tile.TileContext(nc) as tc, ExitStack() as ctx:
        qtp = ctx.enter_context(tc.tile_pool(name="qtp", bufs=3))
        zrp = ctx.enter_context(tc.tile_pool(name="zrp", bufs=12))
        kbp = ctx.enter_context(tc.tile_pool(name="kbp", bufs=6))
        kttp = ctx.enter_context(tc.tile_pool(name="kttp", bufs=4))
        expp = ctx.enter_context(tc.tile_pool(name="expp", bufs=2))
        attp = ctx.enter_context(tc.tile_pool(name="attp", bufs=2))
        atTp = ctx.enter_context(tc.tile_pool(name="atTp", bufs=2))
        statp = ctx.enter_context(tc.tile_pool(name="statp", bufs=8))
        singles = ctx.enter_context(tc.tile_pool(name="singles", bufs=1))
        engp = ctx.enter_context(tc.tile_pool(name="engp", bufs=2, space="PSUM"))
        trp = ctx.enter_context(tc.tile_pool(name="trp", bufs=4, space="PSUM"))
        outp = ctx.enter_context(tc.tile_pool(name="outp", bufs=2, space="PSUM"))

        ident = singles.tile([P, P], BF16)
        make_identity(nc, ident)
        beta_sb = singles.tile([P, 1], F32)
        nc.gpsimd.dma_start(out=beta_sb, in_=beta_ext.ap().to_broadcast((P, 1)))

        deferred_stores = []
        for b in range(BL):
            # ---- interleaved load/compute pipeline: chunk i of the
            # energy matmul consumes zr quarter i (via the kb downcast and
            # PE transposes) and qt chunk i, so the sync-ring order
            # [zr(.,qi), qt_i] feeds compute just-in-time ----
            zr_tiles = {}
            kb = {}
            eng = [engp.tile([P, C], F32, name="eng") for _ in range(CT)]
            for i in range(NCH - 1):
                h, qq = divmod(i, 2)
                for cj in range(CT):
                    zt = zrp.tile([P, NQ], F32)
                    nc.sync.dma_start(
                        out=zt,
                        in_=zr_ext[b, cj * P:(cj + 1) * P, i * NQ:(i + 1) * NQ],
                    )
                    zr_tiles[cj, i] = zt
                for cj in range(CT):
                    if qq == 0:
                        kb[cj, h] = kbp.tile([P, NH], BF16, name="kb_t")
                    nc.scalar.copy(out=kb[cj, h][:, qq * NQ:(qq + 1) * NQ],
                                   in_=zr_tiles[cj, i])
                qt_t = qtp.tile([P, TCH, C], BF16)
                nc.sync.dma_start(out=qt_t, in_=qt_ext[b, :, i * TCH:(i + 1) * TCH, :])
                # transpose+copy producers, then this chunk's matmuls
                ktts = []
                for tg in range(TCH // 4):
                    tr4 = trp.tile([P, 4, CT, P], BF16, name="tr4")
                    for tq in range(4):
                        t = i * TCH + tg * 4 + tq
                        th = t - h * TPH
                        for dj in range(CT):
                            nc.tensor.transpose(tr4[:, tq, dj, :],
                                                kb[dj, h][:, th * P:(th + 1) * P],
                                                ident)
                    ktt4 = kttp.tile([P, 4, CT * P], BF16, name="ktt4")
                    nc.scalar.copy(out=ktt4, in_=tr4)
                    ktts.extend(ktt4[:, tq, :] for tq in range(4))
                for tl in range(TCH // 4 * 4, TCH):
                    t = i * TCH + tl
                    th = t - h * TPH
                    tr2 = trp.tile([P, 4, CT, P], BF16, name="tr2", tag="tr4")
                    for dj in range(CT):
                        nc.tensor.transpose(tr2[:, 0, dj, :],
                                            kb[dj, h][:, th * P:(th + 1) * P],
                                            ident)
                    ktt1 = kttp.tile([P, 4, CT * P], BF16, name="ktt1", tag="ktt4")
                    nc.scalar.copy(out=ktt1[:, 0, :], in_=tr2[:, 0, :, :])
                    ktts.append(ktt1[:, 0, :])
                for tl in range(TCH):
                    t = i * TCH + tl
                    for ci in range(CT):
                        nc.tensor.matmul(
                            eng[ci],
                            lhsT=qt_t[:, tl, ci * P:(ci + 1) * P],
                            rhs=ktts[tl],
                            start=(t == 0),
                            stop=False,
                        )

            # ---- final chunk, split per c-tile: eng[0] closes a full qt
            # sub-load earlier than eng[1], so its softmax / mm2 / stores
            # overlap the ci=1 stream ----
            i = NCH - 1
            h, qq = divmod(i, 2)
            for cj in range(CT):
                zt = zrp.tile([P, NQ], F32)
                nc.sync.dma_start(
                    out=zt,
                    in_=zr_ext[b, cj * P:(cj + 1) * P, i * NQ:(i + 1) * NQ],
                )
                zr_tiles[cj, i] = zt
            for cj in range(CT):
                nc.scalar.copy(out=kb[cj, h][:, qq * NQ:(qq + 1) * NQ],
                               in_=zr_tiles[cj, i])
            ktts = []
            for tg in range(TCH // 4):
                tr4 = trp.tile([P, 4, CT, P], BF16, name="tr4")
                for tq in range(4):
                    t = i * TCH + tg * 4 + tq
                    th = t - h * TPH
                    for dj in range(CT):
                        nc.tensor.transpose(tr4[:, tq, dj, :],
                                            kb[dj, h][:, th * P:(th + 1) * P],
                                            ident)
                ktt4 = kttp.tile([P, 4, CT * P], BF16, name="ktt4")
                nc.scalar.copy(out=ktt4, in_=tr4)
                ktts.extend(ktt4[:, tq, :] for tq in range(4))
            for tl in range(TCH // 4 * 4, TCH):
                t = i * TCH + tl
                th = t - h * TPH
                tr2 = trp.tile([P, 4, CT, P], BF16, name="tr2", tag="tr4")
                for dj in range(CT):
                    nc.tensor.transpose(tr2[:, 0, dj, :],
                                        kb[dj, h][:, th * P:(th + 1) * P],
                                        ident)
                ktt1 = kttp.tile([P, 4, CT * P], BF16, name="ktt1", tag="ktt4")
                nc.scalar.copy(out=ktt1[:, 0, :], in_=tr2[:, 0, :, :])
                ktts.append(ktt1[:, 0, :])
            for ci in range(CT):
                qtt_t = qtp.tile([P, TCH, P], BF16, name="qtt_t", tag="qt_t")
                nc.sync.dma_start(out=qtt_t, in_=qtt_ext[b, ci])
                for tl in range(TCH):
                    t = i * TCH + tl
                    nc.tensor.matmul(
                        eng[ci],
                        lhsT=qtt_t[:, tl, :],
                        rhs=ktts[tl],
                        start=False,
                        stop=(t == NT - 1),
                    )

            if b == BL - 1:
                for dst_ap, src_t in deferred_stores:
                    nc.sync.dma_start(out=dst_ap, in_=src_t)
                deferred_stores = []

            # ---- softmax(max-e) = exp(min-e)/sum; fold beta/sum in.
            # Per-ci attnT tiles keep mm2(ci=0) independent of softmax(1) ----
            attnT = []
            for ci in range(CT):
                mn = statp.tile([P, 1], F32)
                nc.vector.tensor_reduce(out=mn, in_=eng[ci],
                                        axis=mybir.AxisListType.X,
                                        op=mybir.AluOpType.min)
                ex = expp.tile([P, C], F32)
                sm = statp.tile([P, 1], F32)
                nc.scalar.activation(out=ex, in_=eng[ci],
                                     func=mybir.ActivationFunctionType.Exp,
                                     bias=mn, scale=-1.0, accum_out=sm)
                rc = statp.tile([P, 1], F32)
                nc.vector.reciprocal(out=rc, in_=sm)
                rb = statp.tile([P, 1], F32)
                nc.vector.tensor_mul(out=rb, in0=rc, in1=beta_sb)
                at = attp.tile([P, C], BF16)
                nc.vector.tensor_scalar_mul(out=at, in0=ex, scalar1=rb)
                trA = trp.tile([P, CT, P], BF16, name="trA", tag="tr4")
                for dj in range(CT):
                    nc.tensor.transpose(trA[:, dj, :],
                                        at[:, dj * P:(dj + 1) * P], ident)
                atT = atTp.tile([P, CT, P], BF16, name="atT")
                nc.vector.tensor_copy(out=atT, in_=trA)
                attnT.append(atT)

            # ---- out = attn @ k, blended in place into zr, streamed out ----
            # h-outer so the n-low half's stores launch while later work
            # streams; each 4608-wide slice is stored in two 2304-wide pieces
            for ci in range(CT):
                for h in range(2):
                    for qq in range(2):
                        q = h * 2 + qq
                        zt = zr_tiles[ci, q]
                        for wq in range(WPH // 2):
                            w = qq * (WPH // 2) + wq
                            ps = outp.tile([P, OW], F32)
                            for dj in range(CT):
                                nc.tensor.matmul(
                                    ps,
                                    lhsT=attnT[ci][:, dj, :],
                                    rhs=kb[dj, h][:, w * OW:(w + 1) * OW],
                                    start=(dj == 0),
                                    stop=(dj == CT - 1),
                                )
                            nc.vector.tensor_add(
                                out=zt[:, wq * OW:(wq + 1) * OW],
                                in0=ps,
                                in1=zt[:, wq * OW:(wq + 1) * OW])
                        if b == BL - 1:
                            # final batch: 768-wide store pieces so the
                            # store stream starts ~2 blends earlier
                            for hp in range(3):
                                w_ = NQ // 3
                                nc.sync.dma_start(
                                    out=out_ext[b, ci * P:(ci + 1) * P,
                                                q * NQ + hp * w_:
                                                q * NQ + (hp + 1) * w_],
                                    in_=zt[:, hp * w_:(hp + 1) * w_],
                                )
                        elif ci == 1:
                            # deferred into the final batch's tail: frees
                            # mid-kernel DMA bandwidth for its loads and
                            # fills the pre-store dependency gap
                            deferred_stores.append(
                                (out_ext[b, ci * P:(ci + 1) * P,
                                         q * NQ:(q + 1) * NQ], zt))
                        else:
                            nc.gpsimd.dma_start(
                                out=out_ext[b, ci * P:(ci + 1) * P,
                                            q * NQ:(q + 1) * NQ],
                                in_=zt,
                            )

    nc.compile()
    return nc


_NC_CACHE = None


def _get_program():
    global _NC_CACHE
    if _NC_CACHE is None:
        _NC_CACHE = _build_program()
    return _NC_CACHE


def pack_qt(Z1):
    # bf16 q^T, partition-major: full[b, p, t, c] = q[b, c, t*128+p];
    # t < TQT interleaved-ci ("qt"), the last chunk split per ci ("qtt")
    x = Z1.reshape(B, C, NT, P).astype(ml_dtypes.bfloat16)
    full = x.transpose(0, 3, 2, 1)
    qta = np.ascontiguousarray(full[:, :, :TQT, :])
    qtb = np.ascontiguousarray(
        full[:, :, TQT:, :].reshape(B, P, TCH, CT, P).transpose(0, 3, 1, 2, 4))
    return qta, qtb


def kernel(Z1, Zr, beta):
    Z1 = np.asarray(Z1, dtype=np.float32)
    Zr = np.asarray(Zr, dtype=np.float32)
    beta = np.asarray(beta, dtype=np.float32).reshape(1)

    qta, qtb = pack_qt(Z1)
    zr = np.ascontiguousarray(Zr.reshape(B, C, N))

    in_maps = []
    for i in range(NCORES):
        s = slice(i * BL, (i + 1) * BL)
        in_maps.append({"qt": qta[s], "qtt": qtb[s], "zr": zr[s],
                        "beta": beta})

    nc = _get_program()
    res = run_bass_kernel_spmd(nc, in_maps, list(range(NCORES)))
    out = np.concatenate([r["out"] for r in res.results], axis=0)
    return out.reshape(B, C, H, W)



# revision 2
# speedup vs baseline: 1.1955x; 1.1955x over previous
"""Trainium2 Bass kernel for nn_Cross_Attention (B=16, C=256, H=W=96).

reference:
    q = Z1.reshape(B, C, N); k = Zr.reshape(B, C, N)         # N = H*W
    energy    = q @ k^T                                       # [B, C, C]
    attention = softmax(rowmax(energy) - energy, axis=-1)
    out       = attention @ k                                 # [B, C, N]
    return beta * out + Zr
ref absmax ~5.4, tol 2e-2 -> bf16 I/O rounding (~4e-3) is well inside it.

Strategy: data-parallel over batch, 2 batches per NeuronCore on 8 cores.
All HBM I/O in bf16: q^T host-packed [P, NT, C] partition-major so the
N-contraction matmul streams straight from DRAM, Zr host-downcast to bf16
(it is k, the residual, and the mm2 rhs all at once), and the output is
stored bf16 then upcast on host.  k^T for the energy matmul is produced
on-chip tile-by-tile on the TensorE (transpose-mode matmul) so k crosses
HBM exactly once.  softmax(max - e) == exp(min - e) / sum(exp(min - e))
row-wise: only a row-min is needed, exp args are always <= 0 (no
overflow), the sum is >= 1 (no div-by-0).  beta and 1/sum are folded into
the attention weights before the second matmul, so the final blend is a
single bf16 add with the resident k tiles (output == bf16(Zr) when
beta == 0).
"""

from contextlib import ExitStack

import ml_dtypes
import numpy as np

import concourse.bass as bass
import concourse.tile as tile
from concourse import bacc, mybir
from concourse.bass_utils import run_bass_kernel_spmd
from concourse.masks import make_identity

B, C, H, W = 16, 256, 96, 96
N = H * W                    # 9216
P = 128
NCORES = 8
BL = B // NCORES             # 2 batches per core
CT = C // P                  # 2 c-tiles of 128
NT = N // P                  # 72 contraction tiles for energy
TCH = 18                     # qt tiles per DMA chunk
NCH = NT // TCH              # 4 chunks (last one split per c-tile)
TQT = (NCH - 1) * TCH        # 54 t-tiles in the interleaved qt tensor
NH = N // 2                  # 4608: kb slice width (half a c-tile row)
NQ = N // 4                  # 2304: zr tile width (quarter c-tile row)
TPH = NH // P                # 36 n-tiles per h-half
OW = 384                     # mm2 psum chunk width (6 per zr quarter)
WPH = NH // OW               # 12 psum chunks per h-half

F32 = mybir.dt.float32
BF16 = mybir.dt.bfloat16


def _build_program():
    nc = bacc.Bacc("TRN2", target_bir_lowering=False, debug=False,
                   num_devices=NCORES)

    qt_ext = nc.dram_tensor("qt", [BL, P, TQT, C], BF16, kind="ExternalInput")
    qtt_ext = nc.dram_tensor("qtt", [BL, CT, P, TCH, P], BF16,
                             kind="ExternalInput")
    zr_ext = nc.dram_tensor("zr", [BL, C, N], BF16, kind="ExternalInput")
    beta_ext = nc.dram_tensor("beta", [1], F32, kind="ExternalInput")
    out_ext = nc.dram_tensor("out", [BL, C, N], BF16, kind="ExternalOutput")

    with tile.TileContext(nc) as tc, ExitStack() as ctx:
        qtp = ctx.enter_context(tc.tile_pool(name="qtp", bufs=3))
        kbp = ctx.enter_context(tc.tile_pool(name="kbp", bufs=6))
        kttp = ctx.enter_context(tc.tile_pool(name="kttp", bufs=4))
        expp = ctx.enter_context(tc.tile_pool(name="expp", bufs=2))
        attp = ctx.enter_context(tc.tile_pool(name="attp", bufs=2))
        atTp = ctx.enter_context(tc.tile_pool(name="atTp", bufs=2))
        outbp = ctx.enter_context(tc.tile_pool(name="outbp", bufs=4))
        statp = ctx.enter_context(tc.tile_pool(name="statp", bufs=8))
        singles = ctx.enter_context(tc.tile_pool(name="singles", bufs=1))
        engp = ctx.enter_context(tc.tile_pool(name="engp", bufs=2, space="PSUM"))
        trp = ctx.enter_context(tc.tile_pool(name="trp", bufs=4, space="PSUM"))
        outp = ctx.enter_context(tc.tile_pool(name="outp", bufs=2, space="PSUM"))

        ident = singles.tile([P, P], BF16)
        make_identity(nc, ident)
        beta_sb = singles.tile([P, 1], F32)
        nc.gpsimd.dma_start(out=beta_sb, in_=beta_ext.ap().to_broadcast((P, 1)))

        for b in range(BL):
            # ---- interleaved load/compute pipeline: chunk i of the
            # energy matmul consumes zr quarter i (straight into the kb
            # halves, bf16 on the wire) and qt chunk i, so the sync-ring
            # order [zr(.,qi), qt_i] feeds compute just-in-time ----
            kb = {}
            eng = [engp.tile([P, C], F32, name="eng") for _ in range(CT)]
            for i in range(NCH - 1):
                h, qq = divmod(i, 2)
                for cj in range(CT):
                    if qq == 0:
                        kb[cj, h] = kbp.tile([P, NH], BF16, name="kb_t")
                    nc.sync.dma_start(
                        out=kb[cj, h][:, qq * NQ:(qq + 1) * NQ],
                        in_=zr_ext[b, cj * P:(cj + 1) * P, i * NQ:(i + 1) * NQ],
                    )
                qt_t = qtp.tile([P, TCH, C], BF16)
                nc.sync.dma_start(out=qt_t, in_=qt_ext[b, :, i * TCH:(i + 1) * TCH, :])
                # transpose+copy producers, then this chunk's matmuls
                ktts = []
                for tg in range(TCH // 4):
                    tr4 = trp.tile([P, 4, CT, P], BF16, name="tr4")
                    for tq in range(4):
                        t = i * TCH + tg * 4 + tq
                        th = t - h * TPH
                        for dj in range(CT):
                            nc.tensor.transpose(tr4[:, tq, dj, :],
                                                kb[dj, h][:, th * P:(th + 1) * P],
                                                ident)
                    ktt4 = kttp.tile([P, 4, CT * P], BF16, name="ktt4")
                    nc.scalar.copy(out=ktt4, in_=tr4)
                    ktts.extend(ktt4[:, tq, :] for tq in range(4))
                for tl in range(TCH // 4 * 4, TCH):
                    t = i * TCH + tl
                    th = t - h * TPH
                    tr2 = trp.tile([P, 4, CT, P], BF16, name="tr2", tag="tr4")
                    for dj in range(CT):
                        nc.tensor.transpose(tr2[:, 0, dj, :],
                                            kb[dj, h][:, th * P:(th + 1) * P],
                                            ident)
                    ktt1 = kttp.tile([P, 4, CT * P], BF16, name="ktt1", tag="ktt4")
                    nc.scalar.copy(out=ktt1[:, 0, :], in_=tr2[:, 0, :, :])
                    ktts.append(ktt1[:, 0, :])
                for tl in range(TCH):
                    t = i * TCH + tl
                    for ci in range(CT):
                        nc.tensor.matmul(
                            eng[ci],
                            lhsT=qt_t[:, tl, ci * P:(ci + 1) * P],
                            rhs=ktts[tl],
                            start=(t == 0),
                            stop=False,
                        )

            # ---- final chunk, split per c-tile: eng[0] closes a full qt
            # sub-load earlier than eng[1], so its softmax / mm2 / stores
            # overlap the ci=1 stream ----
            i = NCH - 1
            h, qq = divmod(i, 2)
            for cj in range(CT):
                nc.sync.dma_start(
                    out=kb[cj, h][:, qq * NQ:(qq + 1) * NQ],
                    in_=zr_ext[b, cj * P:(cj + 1) * P, i * NQ:(i + 1) * NQ],
                )
            ktts = []
            for tg in range(TCH // 4):
                tr4 = trp.tile([P, 4, CT, P], BF16, name="tr4")
                for tq in range(4):
                    t = i * TCH + tg * 4 + tq
                    th = t - h * TPH
                    for dj in range(CT):
                        nc.tensor.transpose(tr4[:, tq, dj, :],
                                            kb[dj, h][:, th * P:(th + 1) * P],
                                            ident)
                ktt4 = kttp.tile([P, 4, CT * P], BF16, name="ktt4")
                nc.scalar.copy(out=ktt4, in_=tr4)
                ktts.extend(ktt4[:, tq, :] for tq in range(4))
            for tl in range(TCH // 4 * 4, TCH):
                t = i * TCH + tl
                th = t - h * TPH
                tr2 = trp.tile([P, 4, CT, P], BF16, name="tr2", tag="tr4")
                for dj in range(CT):
                    nc.tensor.transpose(tr2[:, 0, dj, :],
                                        kb[dj, h][:, th * P:(th + 1) * P],
                                        ident)
                ktt1 = kttp.tile([P, 4, CT * P], BF16, name="ktt1", tag="ktt4")
                nc.scalar.copy(out=ktt1[:, 0, :], in_=tr2[:, 0, :, :])
                ktts.append(ktt1[:, 0, :])
            for ci in range(CT):
                qtt_t = qtp.tile([P, TCH, P], BF16, name="qtt_t", tag="qt_t")
                nc.sync.dma_start(out=qtt_t, in_=qtt_ext[b, ci])
                for tl in range(TCH):
                    t = i * TCH + tl
                    nc.tensor.matmul(
                        eng[ci],
                        lhsT=qtt_t[:, tl, :],
                        rhs=ktts[tl],
                        start=False,
                        stop=(t == NT - 1),
                    )

            # ---- softmax(max-e) = exp(min-e)/sum; fold beta/sum in.
            # Per-ci attnT tiles keep mm2(ci=0) independent of softmax(1) ----
            attnT = []
            for ci in range(CT):
                mn = statp.tile([P, 1], F32)
                nc.vector.tensor_reduce(out=mn, in_=eng[ci],
                                        axis=mybir.AxisListType.X,
                                        op=mybir.AluOpType.min)
                ex = expp.tile([P, C], F32)
                sm = statp.tile([P, 1], F32)
                nc.scalar.activation(out=ex, in_=eng[ci],
                                     func=mybir.ActivationFunctionType.Exp,
                                     bias=mn, scale=-1.0, accum_out=sm)
                rc = statp.tile([P, 1], F32)
                nc.vector.reciprocal(out=rc, in_=sm)
                rb = statp.tile([P, 1], F32)
                nc.vector.tensor_mul(out=rb, in0=rc, in1=beta_sb)
                at = attp.tile([P, C], BF16)
                nc.vector.tensor_scalar_mul(out=at, in0=ex, scalar1=rb)
                trA = trp.tile([P, CT, P], BF16, name="trA", tag="tr4")
                for dj in range(CT):
                    nc.tensor.transpose(trA[:, dj, :],
                                        at[:, dj * P:(dj + 1) * P], ident)
                atT = atTp.tile([P, CT, P], BF16, name="atT")
                nc.vector.tensor_copy(out=atT, in_=trA)
                attnT.append(atT)

            # ---- out = attn @ k + k(residual), blended into a fresh bf16
            # tile, streamed out in 768-wide pieces as soon as each pair of
            # psum chunks is blended ----
            for ci in range(CT):
                for h in range(2):
                    for qq in range(2):
                        q = h * 2 + qq
                        ot = outbp.tile([P, NQ], BF16, name="ot")
                        for wq in range(WPH // 2):
                            w = qq * (WPH // 2) + wq
                            ps = outp.tile([P, OW], F32)
                            for dj in range(CT):
                                nc.tensor.matmul(
                                    ps,
                                    lhsT=attnT[ci][:, dj, :],
                                    rhs=kb[dj, h][:, w * OW:(w + 1) * OW],
                                    start=(dj == 0),
                                    stop=(dj == CT - 1),
                                )
                            nc.vector.tensor_add(
                                out=ot[:, wq * OW:(wq + 1) * OW],
                                in0=ps,
                                in1=kb[ci, h][:, w * OW:(w + 1) * OW])
                            if wq % 2 == 1:
                                # store each 768-wide piece as its blends
                                # complete so the store stream trails the
                                # PE by only one psum chunk
                                hp = wq // 2
                                w_ = 2 * OW
                                nc.sync.dma_start(
                                    out=out_ext[b, ci * P:(ci + 1) * P,
                                                q * NQ + hp * w_:
                                                q * NQ + (hp + 1) * w_],
                                    in_=ot[:, hp * w_:(hp + 1) * w_],
                                )

    nc.compile()
    return nc


_NC_CACHE = None


def _get_program():
    global _NC_CACHE
    if _NC_CACHE is None:
        _NC_CACHE = _build_program()
    return _NC_CACHE


def pack_qt(Z1):
    # bf16 q^T, partition-major: full[b, p, t, c] = q[b, c, t*128+p];
    # t < TQT interleaved-ci ("qt"), the last chunk split per ci ("qtt")
    x = Z1.reshape(B, C, NT, P).astype(ml_dtypes.bfloat16)
    full = x.transpose(0, 3, 2, 1)
    qta = np.ascontiguousarray(full[:, :, :TQT, :])
    qtb = np.ascontiguousarray(
        full[:, :, TQT:, :].reshape(B, P, TCH, CT, P).transpose(0, 3, 1, 2, 4))
    return qta, qtb


def kernel(Z1, Zr, beta):
    Z1 = np.asarray(Z1, dtype=np.float32)
    Zr = np.asarray(Zr, dtype=np.float32)
    beta = np.asarray(beta, dtype=np.float32).reshape(1)

    qta, qtb = pack_qt(Z1)
    zr = np.ascontiguousarray(
        Zr.reshape(B, C, N).astype(ml_dtypes.bfloat16))

    in_maps = []
    for i in range(NCORES):
        s = slice(i * BL, (i + 1) * BL)
        in_maps.append({"qt": qta[s], "qtt": qtb[s], "zr": zr[s],
                        "beta": beta})

    nc = _get_program()
    res = run_bass_kernel_spmd(nc, in_maps, list(range(NCORES)))
    out = np.concatenate([r["out"] for r in res.results], axis=0)
    return out.astype(np.float32).reshape(B, C, H, W)


# revision 6
# speedup vs baseline: 1.2813x; 1.0718x over previous
"""Trainium2 Bass kernel for nn_Cross_Attention (B=16, C=256, H=W=96).

reference:
    q = Z1.reshape(B, C, N); k = Zr.reshape(B, C, N)         # N = H*W
    energy    = q @ k^T                                       # [B, C, C]
    attention = softmax(rowmax(energy) - energy, axis=-1)
    out       = attention @ k                                 # [B, C, N]
    return beta * out + Zr
ref absmax ~5.4, tol 2e-2 -> bf16 I/O rounding (~4e-3) is well inside it.

Strategy: data-parallel over batch, 2 batches per NeuronCore on 8 cores.
All HBM I/O in bf16: q^T host-packed [P, NT, C] partition-major so the
N-contraction matmul streams straight from DRAM, Zr host-downcast to bf16
(it is k, the residual, and the mm2 rhs all at once), and the output is
stored bf16 then upcast on host.  k^T for the energy matmul is produced
on-chip tile-by-tile on the TensorE (transpose-mode matmul) so k crosses
HBM exactly once.  softmax(max - e) == exp(min - e) / sum(exp(min - e))
row-wise: only a row-min is needed, exp args are always <= 0 (no
overflow), the sum is >= 1 (no div-by-0).  The residual Zr IS k, so
beta*out + Zr == (beta*attention + I) @ k: beta and 1/sum are folded into
the attention weights and I is added to their diagonal block, making the
second matmul produce the final output directly in PSUM (bitwise bf16(Zr)
when beta == 0).  PSUM->SBUF downcast copies alternate between the DVE
and Activation engines so neither gates the TensorE; batch b's stores are
queued on the sync ring BEHIND batch b+1's loads so the store burst never
steals DMA bandwidth from the load stream that feeds the PE.
"""

from contextlib import ExitStack

import ml_dtypes
import numpy as np

import concourse.bass as bass
import concourse.tile as tile
from concourse import bacc, mybir
from concourse.bass_utils import run_bass_kernel_spmd
from concourse.masks import make_identity

B, C, H, W = 16, 256, 96, 96
N = H * W                    # 9216
P = 128
NCORES = 8
BL = B // NCORES             # 2 batches per core
CT = C // P                  # 2 c-tiles of 128
NT = N // P                  # 72 contraction tiles for energy
TCH = 18                     # qt tiles per DMA chunk
NCH = NT // TCH              # 4 chunks (last one split per c-tile)
TQT = (NCH - 1) * TCH        # 54 t-tiles in the interleaved qt tensor
NH = N // 2                  # 4608: kb slice width (half a c-tile row)
NQ = N // 4                  # 2304: zr tile width (quarter c-tile row)
TPH = NH // P                # 36 n-tiles per h-half
OW = 512                     # mm2 psum chunk width == one full PSUM bank
WPH = NH // OW               # 9 psum chunks per h-half
SW = 3 * OW                  # 1536: store piece width (3 per h-half)

F32 = mybir.dt.float32
BF16 = mybir.dt.bfloat16


def _build_program():
    nc = bacc.Bacc("TRN2", target_bir_lowering=False, debug=False,
                   num_devices=NCORES)

    qt_ext = nc.dram_tensor("qt", [BL, P, TQT, C], BF16, kind="ExternalInput")
    qtt_ext = nc.dram_tensor("qtt", [BL, CT, P, TCH, P], BF16,
                             kind="ExternalInput")
    zr_ext = nc.dram_tensor("zr", [BL, C, N], BF16, kind="ExternalInput")
    beta_ext = nc.dram_tensor("beta", [1], F32, kind="ExternalInput")
    out_ext = nc.dram_tensor("out", [BL, C, N], BF16, kind="ExternalOutput")

    with tile.TileContext(nc) as tc, ExitStack() as ctx:
        qtp = ctx.enter_context(tc.tile_pool(name="qtp", bufs=4))
        kbp = ctx.enter_context(tc.tile_pool(name="kbp", bufs=8))
        kttp = ctx.enter_context(tc.tile_pool(name="kttp", bufs=4))
        expp = ctx.enter_context(tc.tile_pool(name="expp", bufs=2))
        attp = ctx.enter_context(tc.tile_pool(name="attp", bufs=2))
        atTp = ctx.enter_context(tc.tile_pool(name="atTp", bufs=2))
        outbp = ctx.enter_context(tc.tile_pool(name="outbp", bufs=6))
        statp = ctx.enter_context(tc.tile_pool(name="statp", bufs=8))
        singles = ctx.enter_context(tc.tile_pool(name="singles", bufs=1))
        engp = ctx.enter_context(tc.tile_pool(name="engp", bufs=2, space="PSUM"))
        trp = ctx.enter_context(tc.tile_pool(name="trp", bufs=4, space="PSUM"))
        outp = ctx.enter_context(tc.tile_pool(name="outp", bufs=2, space="PSUM"))

        ident = singles.tile([P, P], BF16)
        make_identity(nc, ident)
        beta_sb = singles.tile([P, 1], F32)
        nc.gpsimd.dma_start(out=beta_sb, in_=beta_ext.ap().to_broadcast((P, 1)))

        deferred_stores = []
        for b in range(BL):
            # ---- interleaved load/compute pipeline: chunk i of the
            # energy matmul consumes zr quarter i (straight into the kb
            # halves, bf16 on the wire) and qt chunk i (in two half loads
            # so matmuls start before the whole chunk lands) ----
            kb = {}
            eng = [engp.tile([P, C], F32, name="eng") for _ in range(CT)]
            for i in range(NCH - 1):
                h, qq = divmod(i, 2)
                for cj in range(CT):
                    if qq == 0:
                        kb[cj, h] = kbp.tile([P, NH], BF16, name="kb_t")
                    nc.sync.dma_start(
                        out=kb[cj, h][:, qq * NQ:(qq + 1) * NQ],
                        in_=zr_ext[b, cj * P:(cj + 1) * P, i * NQ:(i + 1) * NQ],
                    )
                qt_t = qtp.tile([P, TCH, C], BF16)
                hf = TCH // 2
                nc.sync.dma_start(out=qt_t[:, :hf, :],
                                  in_=qt_ext[b, :, i * TCH:i * TCH + hf, :])
                nc.sync.dma_start(out=qt_t[:, hf:, :],
                                  in_=qt_ext[b, :, i * TCH + hf:(i + 1) * TCH, :])
                # transpose+copy producers, then this chunk's matmuls
                ktts = []
                for tg in range(TCH // 4):
                    tr4 = trp.tile([P, 4, CT, P], BF16, name="tr4")
                    for tq in range(4):
                        t = i * TCH + tg * 4 + tq
                        th = t - h * TPH
                        for dj in range(CT):
                            nc.tensor.transpose(tr4[:, tq, dj, :],
                                                kb[dj, h][:, th * P:(th + 1) * P],
                                                ident)
                    ktt4 = kttp.tile([P, 4, CT * P], BF16, name="ktt4")
                    nc.scalar.copy(out=ktt4, in_=tr4)
                    ktts.extend(ktt4[:, tq, :] for tq in range(4))
                for tl in range(TCH // 4 * 4, TCH):
                    t = i * TCH + tl
                    th = t - h * TPH
                    tr2 = trp.tile([P, 4, CT, P], BF16, name="tr2", tag="tr4")
                    for dj in range(CT):
                        nc.tensor.transpose(tr2[:, 0, dj, :],
                                            kb[dj, h][:, th * P:(th + 1) * P],
                                            ident)
                    ktt1 = kttp.tile([P, 4, CT * P], BF16, name="ktt1", tag="ktt4")
                    nc.scalar.copy(out=ktt1[:, 0, :], in_=tr2[:, 0, :, :])
                    ktts.append(ktt1[:, 0, :])
                for tl in range(TCH):
                    t = i * TCH + tl
                    for ci in range(CT):
                        nc.tensor.matmul(
                            eng[ci],
                            lhsT=qt_t[:, tl, ci * P:(ci + 1) * P],
                            rhs=ktts[tl],
                            start=(t == 0),
                            stop=False,
                        )

            # ---- final chunk, split per c-tile: eng[0] closes a full qt
            # sub-load earlier than eng[1], so its softmax / mm2 / stores
            # overlap the ci=1 stream ----
            i = NCH - 1
            h, qq = divmod(i, 2)
            for cj in range(CT):
                nc.sync.dma_start(
                    out=kb[cj, h][:, qq * NQ:(qq + 1) * NQ],
                    in_=zr_ext[b, cj * P:(cj + 1) * P, i * NQ:(i + 1) * NQ],
                )
            ktts = []
            for tg in range(TCH // 4):
                tr4 = trp.tile([P, 4, CT, P], BF16, name="tr4")
                for tq in range(4):
                    t = i * TCH + tg * 4 + tq
                    th = t - h * TPH
                    for dj in range(CT):
                        nc.tensor.transpose(tr4[:, tq, dj, :],
                                            kb[dj, h][:, th * P:(th + 1) * P],
                                            ident)
                ktt4 = kttp.tile([P, 4, CT * P], BF16, name="ktt4")
                nc.scalar.copy(out=ktt4, in_=tr4)
                ktts.extend(ktt4[:, tq, :] for tq in range(4))
            for tl in range(TCH // 4 * 4, TCH):
                t = i * TCH + tl
                th = t - h * TPH
                tr2 = trp.tile([P, 4, CT, P], BF16, name="tr2", tag="tr4")
                for dj in range(CT):
                    nc.tensor.transpose(tr2[:, 0, dj, :],
                                        kb[dj, h][:, th * P:(th + 1) * P],
                                        ident)
                ktt1 = kttp.tile([P, 4, CT * P], BF16, name="ktt1", tag="ktt4")
                nc.scalar.copy(out=ktt1[:, 0, :], in_=tr2[:, 0, :, :])
                ktts.append(ktt1[:, 0, :])
            for ci in range(CT):
                qtt_t = qtp.tile([P, TCH, P], BF16, name="qtt_t", tag="qt_t")
                nc.sync.dma_start(out=qtt_t, in_=qtt_ext[b, ci])
                for tl in range(TCH):
                    t = i * TCH + tl
                    nc.tensor.matmul(
                        eng[ci],
                        lhsT=qtt_t[:, tl, :],
                        rhs=ktts[tl],
                        start=False,
                        stop=(t == NT - 1),
                    )

            # previous batch's stores, queued on the sync ring BEHIND this
            # batch's loads: they drain in the DMA gap while this batch's
            # mm2 runs, never contending with the loads that feed the PE
            for dst_ap, src_t in deferred_stores:
                nc.sync.dma_start(out=dst_ap, in_=src_t)
            deferred_stores = []

            # ---- softmax(max-e) = exp(min-e)/sum; fold beta/sum in.
            # Per-ci attnT tiles keep mm2(ci=0) independent of softmax(1) ----
            attnT = []
            for ci in range(CT):
                mn = statp.tile([P, 1], F32)
                nc.vector.tensor_reduce(out=mn, in_=eng[ci],
                                        axis=mybir.AxisListType.X,
                                        op=mybir.AluOpType.min)
                ex = expp.tile([P, C], F32)
                sm = statp.tile([P, 1], F32)
                nc.scalar.activation(out=ex, in_=eng[ci],
                                     func=mybir.ActivationFunctionType.Exp,
                                     bias=mn, scale=-1.0, accum_out=sm)
                rc = statp.tile([P, 1], F32)
                nc.vector.reciprocal(out=rc, in_=sm)
                rb = statp.tile([P, 1], F32)
                nc.vector.tensor_mul(out=rb, in0=rc, in1=beta_sb)
                at = attp.tile([P, C], BF16)
                nc.vector.tensor_scalar_mul(out=at, in0=ex, scalar1=rb)
                # residual fold: out = (beta*A + I) @ k, so add I to the
                # diagonal block of this ci's attention rows
                nc.vector.tensor_add(out=at[:, ci * P:(ci + 1) * P],
                                     in0=at[:, ci * P:(ci + 1) * P],
                                     in1=ident)
                trA = trp.tile([P, CT, P], BF16, name="trA", tag="tr4")
                for dj in range(CT):
                    nc.tensor.transpose(trA[:, dj, :],
                                        at[:, dj * P:(dj + 1) * P], ident)
                atT = atTp.tile([P, CT, P], BF16, name="atT")
                nc.vector.tensor_copy(out=atT, in_=trA)
                attnT.append(atT)

            # ---- out = (beta*A + I) @ k: psum holds the final values;
            # downcast copies alternate DVE/Activation, stores stream out
            # in 1536-wide pieces as soon as each 3 chunks are copied ----
            for ci in range(CT):
                for h in range(2):
                    ot = outbp.tile([P, NH], BF16, name="ot")
                    for w in range(WPH):
                        ps = outp.tile([P, OW], F32)
                        for dj in range(CT):
                            nc.tensor.matmul(
                                ps,
                                lhsT=attnT[ci][:, dj, :],
                                rhs=kb[dj, h][:, w * OW:(w + 1) * OW],
                                start=(dj == 0),
                                stop=(dj == CT - 1),
                            )
                        if w % 2 == 0:
                            nc.vector.tensor_copy(
                                out=ot[:, w * OW:(w + 1) * OW], in_=ps)
                        else:
                            nc.scalar.copy(
                                out=ot[:, w * OW:(w + 1) * OW], in_=ps)
                        if w % 3 == 2:
                            seg = w // 3
                            dst = out_ext[b, ci * P:(ci + 1) * P,
                                          h * NH + seg * SW:
                                          h * NH + (seg + 1) * SW]
                            src = ot[:, seg * SW:(seg + 1) * SW]
                            if b < BL - 1:
                                deferred_stores.append((dst, src))
                            else:
                                nc.gpsimd.dma_start(out=dst, in_=src)

    nc.compile()
    return nc


_NC_CACHE = None


def _get_program():
    global _NC_CACHE
    if _NC_CACHE is None:
        _NC_CACHE = _build_program()
    return _NC_CACHE


def pack_qt(Z1):
    # bf16 q^T, partition-major: full[b, p, t, c] = q[b, c, t*128+p];
    # t < TQT interleaved-ci ("qt"), the last chunk split per ci ("qtt")
    x = Z1.reshape(B, C, NT, P).astype(ml_dtypes.bfloat16)
    full = x.transpose(0, 3, 2, 1)
    qta = np.ascontiguousarray(full[:, :, :TQT, :])
    qtb = np.ascontiguousarray(
        full[:, :, TQT:, :].reshape(B, P, TCH, CT, P).transpose(0, 3, 1, 2, 4))
    return qta, qtb


def kernel(Z1, Zr, beta):
    Z1 = np.asarray(Z1, dtype=np.float32)
    Zr = np.asarray(Zr, dtype=np.float32)
    beta = np.asarray(beta, dtype=np.float32).reshape(1)

    qta, qtb = pack_qt(Z1)
    zr = np.ascontiguousarray(
        Zr.reshape(B, C, N).astype(ml_dtypes.bfloat16))

    in_maps = []
    for i in range(NCORES):
        s = slice(i * BL, (i + 1) * BL)
        in_maps.append({"qt": qta[s], "qtt": qtb[s], "zr": zr[s],
                        "beta": beta})

    nc = _get_program()
    res = run_bass_kernel_spmd(nc, in_maps, list(range(NCORES)))
    out = np.concatenate([r["out"] for r in res.results], axis=0)
    return out.astype(np.float32).reshape(B, C, H, W)


# revision 11
# speedup vs baseline: 1.4012x; 1.0936x over previous
"""Trainium2 Bass kernel for nn_Cross_Attention (B=16, C=256, H=W=96).

reference:
    q = Z1.reshape(B, C, N); k = Zr.reshape(B, C, N)         # N = H*W
    energy    = q @ k^T                                       # [B, C, C]
    attention = softmax(rowmax(energy) - energy, axis=-1)
    out       = attention @ k                                 # [B, C, N]
    return beta * out + Zr
ref absmax ~5.4, tol 2e-2 -> bf16 I/O rounding (~4e-3) is well inside it.

Strategy: data-parallel over batch, 2 batches per NeuronCore on 8 cores.
All HBM I/O in bf16: q^T host-packed [P, NT, C] partition-major so the
N-contraction matmul streams straight from DRAM, Zr host-downcast to bf16
(it is k, the residual, and the mm2 rhs all at once), and the output is
stored bf16 then upcast on host.  k^T for the energy matmul is produced
on-chip tile-by-tile on the TensorE (transpose-mode matmul) so k crosses
HBM exactly once.  softmax(max - e) == exp(min - e) / sum(exp(min - e))
row-wise: only a row-min is needed, exp args are always <= 0 (no
overflow), the sum is >= 1 (no div-by-0).  The residual Zr IS k, so
beta*out + Zr == (beta*attention + I) @ k: beta and 1/sum are folded into
the attention weights and I is added to their diagonal block, making the
second matmul produce the final output directly in PSUM (bitwise bf16(Zr)
when beta == 0).  PSUM->SBUF downcast copies alternate between the DVE
and Activation engines so neither gates the TensorE; batch b's stores are
queued on the sync ring BEHIND batch b+1's loads so the store burst never
steals DMA bandwidth from the load stream that feeds the PE.
"""

from contextlib import ExitStack

import ml_dtypes
import numpy as np

import concourse.bass as bass
import concourse.tile as tile
from concourse import bacc, mybir
from concourse.bass_utils import run_bass_kernel_spmd
from concourse.masks import make_identity

B, C, H, W = 16, 256, 96, 96
N = H * W                    # 9216
P = 128
NCORES = 8
BL = B // NCORES             # 2 batches per core
CT = C // P                  # 2 c-tiles of 128
NT = N // P                  # 72 contraction tiles for energy
TCH = 18                     # qt tiles per DMA chunk
NCH = NT // TCH              # 4 chunks (last one split per c-tile)
TQT = (NCH - 1) * TCH        # 54 t-tiles in the interleaved qt tensor
NH = N // 2                  # 4608: kb slice width (half a c-tile row)
NQ = N // 4                  # 2304: zr tile width (quarter c-tile row)
TPH = NH // P                # 36 n-tiles per h-half
OW = 512                     # mm2 psum chunk width == one full PSUM bank
WPH = NH // OW               # 9 psum chunks per h-half
SW = 3 * OW                  # 1536: store piece width (3 per h-half)

F32 = mybir.dt.float32
BF16 = mybir.dt.bfloat16


def _build_program():
    nc = bacc.Bacc("TRN2", target_bir_lowering=False, debug=False,
                   num_devices=NCORES)

    qt_ext = nc.dram_tensor("qt", [BL, P, TQT, C], BF16, kind="ExternalInput")
    qtt_ext = nc.dram_tensor("qtt", [BL, CT, P, TCH, P], BF16,
                             kind="ExternalInput")
    zr_ext = nc.dram_tensor("zr", [BL, C, N], BF16, kind="ExternalInput")
    beta_ext = nc.dram_tensor("beta", [1], F32, kind="ExternalInput")
    out_ext = nc.dram_tensor("out", [BL, C, N], BF16, kind="ExternalOutput")

    with tile.TileContext(nc) as tc, ExitStack() as ctx:
        qtp = ctx.enter_context(tc.tile_pool(name="qtp", bufs=4))
        kbp = ctx.enter_context(tc.tile_pool(name="kbp", bufs=8))
        kttp = ctx.enter_context(tc.tile_pool(name="kttp", bufs=4))
        expp = ctx.enter_context(tc.tile_pool(name="expp", bufs=2))
        attp = ctx.enter_context(tc.tile_pool(name="attp", bufs=2))
        atTp = ctx.enter_context(tc.tile_pool(name="atTp", bufs=2))
        outbp = ctx.enter_context(tc.tile_pool(name="outbp", bufs=6))
        statp = ctx.enter_context(tc.tile_pool(name="statp", bufs=8))
        singles = ctx.enter_context(tc.tile_pool(name="singles", bufs=1))
        engp = ctx.enter_context(tc.tile_pool(name="engp", bufs=2, space="PSUM"))
        trp = ctx.enter_context(tc.tile_pool(name="trp", bufs=3, space="PSUM"))
        outp = ctx.enter_context(tc.tile_pool(name="outp", bufs=3, space="PSUM"))

        ident = singles.tile([P, P], BF16)
        make_identity(nc, ident)
        beta_sb = singles.tile([P, 1], F32)
        nc.gpsimd.dma_start(out=beta_sb, in_=beta_ext.ap().to_broadcast((P, 1)))

        # tl index groups per chunk: [0-3],[4-7],[8-11],[12-15],[16-17] --
        # each fits one 2KB psum bank; copies alternate Activation/DVE so
        # neither engine's serial copy stream gates the energy matmuls
        GROUPS = [list(range(g * 4, min(g * 4 + 4, TCH)))
                  for g in range((TCH + 3) // 4)]

        def emit_tr_group(i, h, g, kb, ktts):
            tls = GROUPS[g]
            tr = trp.tile([P, 4, CT, P], BF16, name="tr4")
            for j, tl in enumerate(tls):
                th = i * TCH + tl - h * TPH
                for dj in range(CT):
                    nc.tensor.transpose(tr[:, j, dj, :],
                                        kb[dj, h][:, th * P:(th + 1) * P],
                                        ident)
            ktt = kttp.tile([P, 4, CT * P], BF16, name="ktt4")
            n = len(tls)
            if (g + i) % 2 == 0:
                nc.scalar.copy(out=ktt[:, :n, :], in_=tr[:, :n, :, :])
            else:
                nc.vector.tensor_copy(out=ktt[:, :n, :], in_=tr[:, :n, :, :])
            for j, tl in enumerate(tls):
                ktts[tl] = ktt[:, j, :]

        deferred_stores = []
        for b in range(BL):
            # ---- interleaved load/compute pipeline: chunk i of the
            # energy matmul consumes zr quarter i (straight into the kb
            # halves, bf16 on the wire) and qt chunk i (in two half loads
            # so matmuls start before the whole chunk lands) ----
            kb = {}
            eng = [engp.tile([P, C], F32, name="eng") for _ in range(CT)]
            for i in range(NCH - 1):
                h, qq = divmod(i, 2)
                for cj in range(CT):
                    if qq == 0:
                        kb[cj, h] = kbp.tile([P, NH], BF16, name="kb_t")
                    nc.sync.dma_start(
                        out=kb[cj, h][:, qq * NQ:(qq + 1) * NQ],
                        in_=zr_ext[b, cj * P:(cj + 1) * P, i * NQ:(i + 1) * NQ],
                    )
                qt_t = qtp.tile([P, TCH, C], BF16)
                hf = TCH // 2
                nc.sync.dma_start(out=qt_t[:, :hf, :],
                                  in_=qt_ext[b, :, i * TCH:i * TCH + hf, :])
                nc.sync.dma_start(out=qt_t[:, hf:, :],
                                  in_=qt_ext[b, :, i * TCH + hf:(i + 1) * TCH, :])
                # pipelined transpose-group / matmul-group emission with a
                # lookahead of 2, so only 3 psum transpose bufs are live
                # and the PE always has matmul work while copies land
                ktts = [None] * TCH
                emit_tr_group(i, h, 0, kb, ktts)
                emit_tr_group(i, h, 1, kb, ktts)
                for g in range(len(GROUPS)):
                    if g + 2 < len(GROUPS):
                        emit_tr_group(i, h, g + 2, kb, ktts)
                    for tl in GROUPS[g]:
                        t = i * TCH + tl
                        for ci in range(CT):
                            nc.tensor.matmul(
                                eng[ci],
                                lhsT=qt_t[:, tl, ci * P:(ci + 1) * P],
                                rhs=ktts[tl],
                                start=(t == 0),
                                stop=False,
                            )

            # ---- final chunk, split per c-tile: eng[0] closes a full qt
            # sub-load earlier than eng[1], so its softmax / mm2 / stores
            # overlap the ci=1 stream ----
            i = NCH - 1
            h, qq = divmod(i, 2)
            for cj in range(CT):
                nc.sync.dma_start(
                    out=kb[cj, h][:, qq * NQ:(qq + 1) * NQ],
                    in_=zr_ext[b, cj * P:(cj + 1) * P, i * NQ:(i + 1) * NQ],
                )
            ktts = [None] * TCH
            qtt_ts = []
            for ci in range(CT):
                qtt_t = qtp.tile([P, TCH, P], BF16, name="qtt_t", tag="qt_t")
                nc.sync.dma_start(out=qtt_t, in_=qtt_ext[b, ci])
                qtt_ts.append(qtt_t)
            emit_tr_group(i, h, 0, kb, ktts)
            emit_tr_group(i, h, 1, kb, ktts)
            for g in range(len(GROUPS)):
                if g + 2 < len(GROUPS):
                    emit_tr_group(i, h, g + 2, kb, ktts)
                for tl in GROUPS[g]:
                    t = i * TCH + tl
                    nc.tensor.matmul(
                        eng[0],
                        lhsT=qtt_ts[0][:, tl, :],
                        rhs=ktts[tl],
                        start=False,
                        stop=(t == NT - 1),
                    )
            for tl in range(TCH):
                t = i * TCH + tl
                nc.tensor.matmul(
                    eng[1],
                    lhsT=qtt_ts[1][:, tl, :],
                    rhs=ktts[tl],
                    start=False,
                    stop=(t == NT - 1),
                )

            # previous batch's stores, queued on the sync ring BEHIND this
            # batch's loads: they drain in the DMA gap while this batch's
            # mm2 runs, never contending with the loads that feed the PE
            for dst_ap, src_t in deferred_stores:
                nc.sync.dma_start(out=dst_ap, in_=src_t)
            deferred_stores = []

            # ---- softmax(max-e) = exp(min-e)/sum; fold beta/sum in.
            # Per-ci attnT tiles keep mm2(ci=0) independent of softmax(1) ----
            attnT = []
            for ci in range(CT):
                mn = statp.tile([P, 1], F32)
                nc.vector.tensor_reduce(out=mn, in_=eng[ci],
                                        axis=mybir.AxisListType.X,
                                        op=mybir.AluOpType.min)
                ex = expp.tile([P, C], F32)
                sm = statp.tile([P, 1], F32)
                nc.scalar.activation(out=ex, in_=eng[ci],
                                     func=mybir.ActivationFunctionType.Exp,
                                     bias=mn, scale=-1.0, accum_out=sm)
                rc = statp.tile([P, 1], F32)
                nc.vector.reciprocal(out=rc, in_=sm)
                rb = statp.tile([P, 1], F32)
                nc.vector.tensor_mul(out=rb, in0=rc, in1=beta_sb)
                at = attp.tile([P, C], BF16)
                nc.vector.tensor_scalar_mul(out=at, in0=ex, scalar1=rb)
                # residual fold: out = (beta*A + I) @ k, so add I to the
                # diagonal block of this ci's attention rows
                nc.vector.tensor_add(out=at[:, ci * P:(ci + 1) * P],
                                     in0=at[:, ci * P:(ci + 1) * P],
                                     in1=ident)
                trA = trp.tile([P, CT, P], BF16, name="trA", tag="tr4")
                for dj in range(CT):
                    nc.tensor.transpose(trA[:, dj, :],
                                        at[:, dj * P:(dj + 1) * P], ident)
                atT = atTp.tile([P, CT, P], BF16, name="atT")
                nc.vector.tensor_copy(out=atT, in_=trA)
                attnT.append(atT)

            # ---- out = (beta*A + I) @ k: psum holds the final values;
            # downcast copies alternate DVE/Activation, stores stream out
            # in 1536-wide pieces as soon as each 3 chunks are copied ----
            for ci in range(CT):
                for h in range(2):
                    ot = outbp.tile([P, NH], BF16, name="ot")
                    for w in range(WPH):
                        ps = outp.tile([P, OW], F32)
                        for dj in range(CT):
                            nc.tensor.matmul(
                                ps,
                                lhsT=attnT[ci][:, dj, :],
                                rhs=kb[dj, h][:, w * OW:(w + 1) * OW],
                                start=(dj == 0),
                                stop=(dj == CT - 1),
                            )
                        if w % 2 == 0:
                            nc.vector.tensor_copy(
                                out=ot[:, w * OW:(w + 1) * OW], in_=ps)
                        else:
                            nc.scalar.copy(
                                out=ot[:, w * OW:(w + 1) * OW], in_=ps)
                        if w % 3 == 2:
                            seg = w // 3
                            dst = out_ext[b, ci * P:(ci + 1) * P,
                                          h * NH + seg * SW:
                                          h * NH + (seg + 1) * SW]
                            src = ot[:, seg * SW:(seg + 1) * SW]
                            if b < BL - 1:
                                deferred_stores.append((dst, src))
                            else:
                                nc.gpsimd.dma_start(out=dst, in_=src)

    nc.compile()
    return nc


_NC_CACHE = None


def _get_program():
    global _NC_CACHE
    if _NC_CACHE is None:
        _NC_CACHE = _build_program()
    return _NC_CACHE


def pack_qt(Z1):
    # bf16 q^T, partition-major: full[b, p, t, c] = q[b, c, t*128+p];
    # t < TQT interleaved-ci ("qt"), the last chunk split per ci ("qtt")
    x = Z1.reshape(B, C, NT, P).astype(ml_dtypes.bfloat16)
    full = x.transpose(0, 3, 2, 1)
    qta = np.ascontiguousarray(full[:, :, :TQT, :])
    qtb = np.ascontiguousarray(
        full[:, :, TQT:, :].reshape(B, P, TCH, CT, P).transpose(0, 3, 1, 2, 4))
    return qta, qtb


def kernel(Z1, Zr, beta):
    Z1 = np.asarray(Z1, dtype=np.float32)
    Zr = np.asarray(Zr, dtype=np.float32)
    beta = np.asarray(beta, dtype=np.float32).reshape(1)

    qta, qtb = pack_qt(Z1)
    zr = np.ascontiguousarray(
        Zr.reshape(B, C, N).astype(ml_dtypes.bfloat16))

    in_maps = []
    for i in range(NCORES):
        s = slice(i * BL, (i + 1) * BL)
        in_maps.append({"qt": qta[s], "qtt": qtb[s], "zr": zr[s],
                        "beta": beta})

    nc = _get_program()
    res = run_bass_kernel_spmd(nc, in_maps, list(range(NCORES)))
    out = np.concatenate([r["out"] for r in res.results], axis=0)
    return out.astype(np.float32).reshape(B, C, H, W)


# revision 12
# speedup vs baseline: 1.4128x; 1.0083x over previous
"""Trainium2 Bass kernel for nn_Cross_Attention (B=16, C=256, H=W=96).

reference:
    q = Z1.reshape(B, C, N); k = Zr.reshape(B, C, N)         # N = H*W
    energy    = q @ k^T                                       # [B, C, C]
    attention = softmax(rowmax(energy) - energy, axis=-1)
    out       = attention @ k                                 # [B, C, N]
    return beta * out + Zr
ref absmax ~5.4, tol 2e-2 -> bf16 I/O rounding (~4e-3) is well inside it.

Strategy: data-parallel over batch, 2 batches per NeuronCore on 8 cores.
All HBM I/O in bf16: q^T host-packed [P, NT, C] partition-major so the
N-contraction matmul streams straight from DRAM, Zr host-downcast to bf16
(it is k, the residual, and the mm2 rhs all at once), and the output is
stored bf16 then upcast on host.  k^T for the energy matmul is produced
on-chip tile-by-tile on the TensorE (transpose-mode matmul) so k crosses
HBM exactly once.  softmax(max - e) == exp(min - e) / sum(exp(min - e))
row-wise: only a row-min is needed, exp args are always <= 0 (no
overflow), the sum is >= 1 (no div-by-0).  The residual Zr IS k, so
beta*out + Zr == (beta*attention + I) @ k: beta and 1/sum are folded into
the attention weights and I is added to their diagonal block, making the
second matmul produce the final output directly in PSUM (bitwise bf16(Zr)
when beta == 0).  PSUM->SBUF downcast copies alternate between the DVE
and Activation engines so neither gates the TensorE; batch b's stores are
queued on the sync ring BEHIND batch b+1's loads so the store burst never
steals DMA bandwidth from the load stream that feeds the PE.
"""

from contextlib import ExitStack

import ml_dtypes
import numpy as np

import concourse.bass as bass
import concourse.tile as tile
from concourse import bacc, mybir
from concourse.bass_utils import run_bass_kernel_spmd
from concourse.masks import make_identity

B, C, H, W = 16, 256, 96, 96
N = H * W                    # 9216
P = 128
NCORES = 8
BL = B // NCORES             # 2 batches per core
CT = C // P                  # 2 c-tiles of 128
NT = N // P                  # 72 contraction tiles for energy
TCH = 18                     # qt tiles per DMA chunk
NCH = NT // TCH              # 4 chunks (last one split per c-tile)
TQT = (NCH - 1) * TCH        # 54 t-tiles in the interleaved qt tensor
NH = N // 2                  # 4608: kb slice width (half a c-tile row)
NQ = N // 4                  # 2304: zr tile width (quarter c-tile row)
TPH = NH // P                # 36 n-tiles per h-half
OW = 512                     # mm2 psum chunk width == one full PSUM bank
WPH = NH // OW               # 9 psum chunks per h-half
SW = 3 * OW                  # 1536: store piece width (3 per h-half)

F32 = mybir.dt.float32
BF16 = mybir.dt.bfloat16


def _build_program():
    nc = bacc.Bacc("TRN2", target_bir_lowering=False, debug=False,
                   num_devices=NCORES)

    qt_ext = nc.dram_tensor("qt", [BL, P, TQT, C], BF16, kind="ExternalInput")
    qtt_ext = nc.dram_tensor("qtt", [BL, CT, P, TCH, P], BF16,
                             kind="ExternalInput")
    zr_ext = nc.dram_tensor("zr", [BL, C, N], BF16, kind="ExternalInput")
    beta_ext = nc.dram_tensor("beta", [1], F32, kind="ExternalInput")
    out_ext = nc.dram_tensor("out", [BL, C, N], BF16, kind="ExternalOutput")

    with tile.TileContext(nc) as tc, ExitStack() as ctx:
        qtp = ctx.enter_context(tc.tile_pool(name="qtp", bufs=4))
        kbp = ctx.enter_context(tc.tile_pool(name="kbp", bufs=8))
        kttp = ctx.enter_context(tc.tile_pool(name="kttp", bufs=4))
        expp = ctx.enter_context(tc.tile_pool(name="expp", bufs=2))
        attp = ctx.enter_context(tc.tile_pool(name="attp", bufs=2))
        atTp = ctx.enter_context(tc.tile_pool(name="atTp", bufs=2))
        outbp = ctx.enter_context(tc.tile_pool(name="outbp", bufs=6))
        statp = ctx.enter_context(tc.tile_pool(name="statp", bufs=8))
        singles = ctx.enter_context(tc.tile_pool(name="singles", bufs=1))
        engp = ctx.enter_context(tc.tile_pool(name="engp", bufs=2, space="PSUM"))
        trp = ctx.enter_context(tc.tile_pool(name="trp", bufs=3, space="PSUM"))
        outp = ctx.enter_context(tc.tile_pool(name="outp", bufs=3, space="PSUM"))

        ident = singles.tile([P, P], BF16)
        make_identity(nc, ident)
        beta_sb = singles.tile([P, 1], F32)
        nc.gpsimd.dma_start(out=beta_sb, in_=beta_ext.ap().to_broadcast((P, 1)))

        # tl index groups per chunk: [0-3],[4-7],[8-11],[12-15],[16-17] --
        # each fits one 2KB psum bank; copies alternate Activation/DVE so
        # neither engine's serial copy stream gates the energy matmuls
        GROUPS = [list(range(g * 4, min(g * 4 + 4, TCH)))
                  for g in range((TCH + 3) // 4)]

        def emit_tr_group(i, h, g, kb, ktts):
            tls = GROUPS[g]
            tr = trp.tile([P, 4, CT, P], BF16, name="tr4")
            for j, tl in enumerate(tls):
                th = i * TCH + tl - h * TPH
                for dj in range(CT):
                    nc.tensor.transpose(tr[:, j, dj, :],
                                        kb[dj, h][:, th * P:(th + 1) * P],
                                        ident)
            ktt = kttp.tile([P, 4, CT * P], BF16, name="ktt4")
            n = len(tls)
            if (g + i) % 2 == 0:
                nc.scalar.copy(out=ktt[:, :n, :], in_=tr[:, :n, :, :])
            else:
                nc.vector.tensor_copy(out=ktt[:, :n, :], in_=tr[:, :n, :, :])
            for j, tl in enumerate(tls):
                ktts[tl] = ktt[:, j, :]

        deferred_stores = []
        for b in range(BL):
            # ---- interleaved load/compute pipeline: chunk i of the
            # energy matmul consumes zr quarter i (straight into the kb
            # halves, bf16 on the wire) and qt chunk i (in two half loads
            # so matmuls start before the whole chunk lands) ----
            kb = {}
            eng = [engp.tile([P, C], F32, name="eng") for _ in range(CT)]
            for i in range(NCH - 1):
                h, qq = divmod(i, 2)
                for cj in range(CT):
                    if qq == 0:
                        kb[cj, h] = kbp.tile([P, NH], BF16, name="kb_t")
                    nc.sync.dma_start(
                        out=kb[cj, h][:, qq * NQ:(qq + 1) * NQ],
                        in_=zr_ext[b, cj * P:(cj + 1) * P, i * NQ:(i + 1) * NQ],
                    )
                qt_t = qtp.tile([P, TCH, C], BF16)
                hf = TCH // 2
                nc.sync.dma_start(out=qt_t[:, :hf, :],
                                  in_=qt_ext[b, :, i * TCH:i * TCH + hf, :])
                nc.sync.dma_start(out=qt_t[:, hf:, :],
                                  in_=qt_ext[b, :, i * TCH + hf:(i + 1) * TCH, :])
                # pipelined transpose-group / matmul-group emission with a
                # lookahead of 2, so only 3 psum transpose bufs are live
                # and the PE always has matmul work while copies land
                ktts = [None] * TCH
                emit_tr_group(i, h, 0, kb, ktts)
                emit_tr_group(i, h, 1, kb, ktts)
                for g in range(len(GROUPS)):
                    if g + 2 < len(GROUPS):
                        emit_tr_group(i, h, g + 2, kb, ktts)
                    for tl in GROUPS[g]:
                        t = i * TCH + tl
                        for ci in range(CT):
                            nc.tensor.matmul(
                                eng[ci],
                                lhsT=qt_t[:, tl, ci * P:(ci + 1) * P],
                                rhs=ktts[tl],
                                start=(t == 0),
                                stop=False,
                            )

            # ---- final chunk, split per c-tile: eng[0] closes a full qt
            # sub-load earlier than eng[1], so its softmax / mm2 / stores
            # overlap the ci=1 stream ----
            i = NCH - 1
            h, qq = divmod(i, 2)
            for cj in range(CT):
                nc.sync.dma_start(
                    out=kb[cj, h][:, qq * NQ:(qq + 1) * NQ],
                    in_=zr_ext[b, cj * P:(cj + 1) * P, i * NQ:(i + 1) * NQ],
                )
            ktts = [None] * TCH
            qtt_ts = []
            for ci in range(CT):
                qtt_t = qtp.tile([P, TCH, P], BF16, name="qtt_t", tag="qt_t")
                nc.sync.dma_start(out=qtt_t, in_=qtt_ext[b, ci])
                qtt_ts.append(qtt_t)
            emit_tr_group(i, h, 0, kb, ktts)
            emit_tr_group(i, h, 1, kb, ktts)
            for g in range(len(GROUPS)):
                if g + 2 < len(GROUPS):
                    emit_tr_group(i, h, g + 2, kb, ktts)
                for tl in GROUPS[g]:
                    t = i * TCH + tl
                    nc.tensor.matmul(
                        eng[0],
                        lhsT=qtt_ts[0][:, tl, :],
                        rhs=ktts[tl],
                        start=False,
                        stop=(t == NT - 1),
                    )
            for tl in range(TCH):
                t = i * TCH + tl
                nc.tensor.matmul(
                    eng[1],
                    lhsT=qtt_ts[1][:, tl, :],
                    rhs=ktts[tl],
                    start=False,
                    stop=(t == NT - 1),
                )

            # previous batch's stores, queued on the sync ring BEHIND this
            # batch's loads: they drain in the DMA gap while this batch's
            # mm2 runs, never contending with the loads that feed the PE
            for dst_ap, src_t in deferred_stores:
                nc.sync.dma_start(out=dst_ap, in_=src_t)
            deferred_stores = []

            # ---- softmax(max-e) = exp(min-e)/sum; fold beta/sum in.
            # Per-ci attnT tiles keep mm2(ci=0) independent of softmax(1) ----
            attnT = []
            for ci in range(CT):
                mn = statp.tile([P, 1], F32)
                nc.vector.tensor_reduce(out=mn, in_=eng[ci],
                                        axis=mybir.AxisListType.X,
                                        op=mybir.AluOpType.min)
                ex = expp.tile([P, C], F32)
                sm = statp.tile([P, 1], F32)
                nc.scalar.activation(out=ex, in_=eng[ci],
                                     func=mybir.ActivationFunctionType.Exp,
                                     bias=mn, scale=-1.0, accum_out=sm)
                rc = statp.tile([P, 1], F32)
                nc.vector.reciprocal(out=rc, in_=sm)
                rb = statp.tile([P, 1], F32)
                nc.vector.tensor_mul(out=rb, in0=rc, in1=beta_sb)
                at = attp.tile([P, C], BF16)
                nc.vector.tensor_scalar_mul(out=at, in0=ex, scalar1=rb)
                # residual fold: out = (beta*A + I) @ k, so add I to the
                # diagonal block of this ci's attention rows
                nc.vector.tensor_add(out=at[:, ci * P:(ci + 1) * P],
                                     in0=at[:, ci * P:(ci + 1) * P],
                                     in1=ident)
                trA = trp.tile([P, CT, P], BF16, name="trA", tag="tr4")
                for dj in range(CT):
                    nc.tensor.transpose(trA[:, dj, :],
                                        at[:, dj * P:(dj + 1) * P], ident)
                atT = atTp.tile([P, CT, P], BF16, name="atT")
                nc.vector.tensor_copy(out=atT, in_=trA)
                attnT.append(atT)

            # ---- out = (beta*A + I) @ k: psum holds the final values;
            # downcast copies alternate DVE/Activation, stores stream out
            # in 1536-wide pieces as soon as each 3 chunks are copied ----
            for ci in range(CT):
                for h in range(2):
                    ot = outbp.tile([P, NH], BF16, name="ot")
                    for w in range(WPH):
                        ps = outp.tile([P, OW], F32)
                        for dj in range(CT):
                            nc.tensor.matmul(
                                ps,
                                lhsT=attnT[ci][:, dj, :],
                                rhs=kb[dj, h][:, w * OW:(w + 1) * OW],
                                start=(dj == 0),
                                stop=(dj == CT - 1),
                            )
                        if w % 2 == 0:
                            nc.vector.tensor_copy(
                                out=ot[:, w * OW:(w + 1) * OW], in_=ps)
                        else:
                            nc.scalar.copy(
                                out=ot[:, w * OW:(w + 1) * OW], in_=ps)
                        if w % 3 == 2:
                            seg = w // 3
                            dst = out_ext[b, ci * P:(ci + 1) * P,
                                          h * NH + seg * SW:
                                          h * NH + (seg + 1) * SW]
                            src = ot[:, seg * SW:(seg + 1) * SW]
                            if b < BL - 1:
                                deferred_stores.append((dst, src))
                            else:
                                nc.sync.dma_start(out=dst, in_=src)

    nc.compile()
    return nc


_NC_CACHE = None


def _get_program():
    global _NC_CACHE
    if _NC_CACHE is None:
        _NC_CACHE = _build_program()
    return _NC_CACHE


def pack_qt(Z1):
    # bf16 q^T, partition-major: full[b, p, t, c] = q[b, c, t*128+p];
    # t < TQT interleaved-ci ("qt"), the last chunk split per ci ("qtt")
    x = Z1.reshape(B, C, NT, P).astype(ml_dtypes.bfloat16)
    full = x.transpose(0, 3, 2, 1)
    qta = np.ascontiguousarray(full[:, :, :TQT, :])
    qtb = np.ascontiguousarray(
        full[:, :, TQT:, :].reshape(B, P, TCH, CT, P).transpose(0, 3, 1, 2, 4))
    return qta, qtb


def kernel(Z1, Zr, beta):
    Z1 = np.asarray(Z1, dtype=np.float32)
    Zr = np.asarray(Zr, dtype=np.float32)
    beta = np.asarray(beta, dtype=np.float32).reshape(1)

    qta, qtb = pack_qt(Z1)
    zr = np.ascontiguousarray(
        Zr.reshape(B, C, N).astype(ml_dtypes.bfloat16))

    in_maps = []
    for i in range(NCORES):
        s = slice(i * BL, (i + 1) * BL)
        in_maps.append({"qt": qta[s], "qtt": qtb[s], "zr": zr[s],
                        "beta": beta})

    nc = _get_program()
    res = run_bass_kernel_spmd(nc, in_maps, list(range(NCORES)))
    out = np.concatenate([r["out"] for r in res.results], axis=0)
    return out.astype(np.float32).reshape(B, C, H, W)
